# revision 26
# baseline (speedup 1.0000x reference)
"""EventRNN (sparse_attention) Trainium2 Bass kernel.

Full-input contract: kernel(**inputs) takes the complete arrays from
setup_inputs() and returns the full (h_new[None], c_new[None]) tuple.

Sharding: data-parallel over batch B=32 across 8 NeuronCores (4 batches
per core); all weights replicated. Host-side prep is layout-only
(transposes / slicing / dtype casts / bool->additive mask).

Design (DMA-roofline focused; per-core DMA 25.8 MB -> 14.0 MB):
 - dtypes: features / features_proj / alpha / w_h2a / w_sel / h ride as
   fp8e4m3. The LSTM weight splits by k-chunk: 4 bf16 chunks (cap), 8
   fp8e3m4 chunks (feature/h, x128 scale), 4 fp8e4m3 chunks (the
   attention-context rows, paired with a x64-scaled fp8 x_fc). Measured
   on HW: rel err 1.57e-2 (gate 2e-2); all-bf16 gives 2.7e-3 but costs
   +8 MB/core.
 - DMA order: proj0 -> packed consts (3 DMAs) -> all proj/feats -> W
   chunks; the fc-W rides last, split into gate-block pieces [i,f], [g],
   [o] so the c_new chain overlaps the final transfers and only the
   sig_o*tanh(c) chain trails the last byte.
 - logits are computed TRANSPOSED: lhsT = 128x128 relu'd-proj chunks
   (stationary), rhs = attention weight column -> logits land in PSUM
   as [128(l), 8(lc)]. Softmax exp runs on ACT at free-size 8, alpha
   needs no PE transposes, no max-subtract (bounded logits, fp32 psum),
   alpha written directly as fp8 for the ctx matvec.
 - relu(proj+q) fans out across Pool/DVE/ACT (dc0/dc1/dc2+3).
 - gates matmul runs output-transposed: lhsT = W.T 128x128 chunks
   (stationary), rhs = xhT [128,4]; three psum tiles (i|f, o, g) so the
   LSTM tail reads each as soon as its own writers finish. Bias lands
   via one K=16 preload matmul per tile (the single start=True for each
   psum bank; 2KB zero-region semantics). LSTM elementwise tail runs in
   the transposed layout; host un-transposes the [128, 32] output.
 - one ACT table for the whole kernel (exp_and_others: relu/exp/
   identity/tanh); sigmoids via 0.5*tanh(x/2)+0.5. No table switches.

TimelineSim projection: 46.0 us (baseline 95.7 us). DMA_ENGINES busy
38.9 us (85%, gapless): lead-in ~2.0 us (init barrier + HWDGE/DGE pipe)
and last-byte tail ~5.1 us (DMA-completion sem, the sig_o chain, output
DMA issue, end-of-kernel drains) are structural; the c_new chain and
sig_o chain converge within ~30 ns of each other at the output.
"""

import numpy as np

import concourse.bacc as bacc
import concourse.mybir as mybir
import concourse.tile as tile
import concourse.masks as masks
from concourse.bass_utils import run_bass_kernel_spmd

F32 = mybir.dt.float32
BF16 = mybir.dt.bfloat16
F8 = mybir.dt.float8e4
F83 = mybir.dt.float8e3
AF = mybir.ActivationFunctionType
ALU = mybir.AluOpType

B, L, D, H = 32, 2048, 512, 512
N_CORES = 8
B_LOC = B // N_CORES          # 4 batches per core
FIDX = 1024                   # static feature_idx from setup_inputs()
HALF = L // 2                 # past/future split == 1024
P = 128
DC = D // P                   # 4 d-chunks
HC = H // P                   # 4 h-chunks
LC = HALF // P                # 8 l-chunks per half
KC = 16                       # k-chunks of the fused LSTM matmul (2048/128)
GC = 16                       # gate-column chunks (4H/128)
G4 = 4 * H
S_W = 1024.0                  # fp8 scale for the small matvec weights
S_G = 128.0                   # global gates scale (W*128, descaled in ACT)
N_E3M4 = 8                    # trailing cap/feat/h W chunks in fp8e3m4
N_BF = 12 - N_E3M4            # leading W chunks kept bf16
# xhT k-chunk order: [cap 0:4 | feature 4:8 | h 8:12 | fc 12:16] so the
# attention-context-dependent chunks come last (their W pair is also the
# last DMA; the gates tail then runs right as the final W bytes land).
FC_KCS = (12, 13, 14, 15)
WT_PAIRS = [(0, 1), (2, 3), (4, 5), (6, 7), (8, 9), (10, 11), (12, 13),
            (14, 15)]
# packed-const offsets
F8C_WSEL = 2048               # f8 pack: w_h2a [0:2048], w_sel, hT8
F8C_HT = 2052
BFC_WPF = 64                  # bf16 pack: xhT [0:64], w_pf [64:72]
SC_BRHS = 128                 # small pack rows=16: biasT2 [0:128], brhs,
SC_ID8 = 192                  # ident8 [192:200], madd2 [200:1224] (rows 0:8)
SC_MADD = 200


def build_nc():
    nc = bacc.Bacc("TRN2", target_bir_lowering=False, debug=False,
                   num_devices=N_CORES)

    # ---- DRAM I/O ----
    projT = nc.dram_tensor("projT", [B_LOC, D, L], F8, kind="ExternalInput").ap()
    feats = nc.dram_tensor("feats", [B_LOC, L, D], F8, kind="ExternalInput").ap()
    WT = nc.dram_tensor("WT", [N_BF * P, G4], BF16, kind="ExternalInput").ap()
    WT83 = nc.dram_tensor("WT83", [N_E3M4 * P, G4], F83,
                          kind="ExternalInput").ap()
    WT8 = nc.dram_tensor("WT8", [4 * P, G4], F8, kind="ExternalInput").ap()
    f8c = nc.dram_tensor("f8c", [P, 2068], F8, kind="ExternalInput").ap()
    f32c = nc.dram_tensor("f32c", [P, 21], F32, kind="ExternalInput").ap()
    bfc = nc.dram_tensor("bfc", [P, 72], BF16, kind="ExternalInput").ap()
    smallc = nc.dram_tensor("smallc", [16, 1224], BF16, kind="ExternalInput").ap()
    out_d = nc.dram_tensor("out", [P, 32], F32, kind="ExternalOutput").ap()

    with tile.TileContext(nc) as tc:
        with tc.tile_pool(name="const", bufs=1) as const, \
             tc.tile_pool(name="wres", bufs=1) as wres:
            # ---- resident constants / packed small inputs ----
            ident = const.tile([P, P], F32)
            ones_f8 = const.tile([P, 1], F8)
            f8c_sb = const.tile([P, 2068], F8)
            f32c_sb = const.tile([P, 21], F32)
            smallc_sb = const.tile([16, 1224], BF16)
            # xhT lives inside the bf16 pack; fc x-chunks go to the fp8 tile
            bfc_sb = const.tile([P, 72], BF16)
            xf8_sb = const.tile([P, 16], F8)

            # resident LSTM weights: bf16 + e3m4 cap/feat/h + e4m3 fc
            WT_sb = wres.tile([P, N_BF, G4], BF16)
            WT83_sb = wres.tile([P, N_E3M4, G4], F83)
            WT8_sb = wres.tile([P, 4, G4], F8)

            # scalars along free dims, r = b*2 + h
            qb = const.tile([P, DC, B_LOC], F32)
            tb = const.tile([1, B_LOC], F32)
            beta_sb = const.tile([1, B_LOC], F32)
            sums_sb = const.tile([1, 2 * B_LOC], F32)
            recips = const.tile([1, 2 * B_LOC], F32)
            svals = const.tile([1, 2 * B_LOC], F32)

            with tc.tile_pool(name="proj", bufs=8) as projp, \
                 tc.tile_pool(name="hatt", bufs=2) as hattp, \
                 tc.tile_pool(name="fpool", bufs=8) as fpool, \
                 tc.tile_pool(name="alphap", bufs=3) as alphap, \
                 tc.tile_pool(name="fcp", bufs=2) as fcp:

                # ---- DMA order: proj0, packed consts, then the stream;
                # bf16 W pairs next-to-last, the fp8 fc W chunk dead last ----
                projts, featsts = [], []
                for u in range(2 * B_LOC):
                    b, h = divmod(u, 2)
                    projt = projp.tile([P, DC, HALF], F8)
                    nc.sync.dma_start(
                        projt[:],
                        projT[b, :, h * HALF:(h + 1) * HALF]
                        .rearrange("(c p) l -> p c l", p=P))
                    projts.append(projt)
                    if u == 0:
                        nc.sync.dma_start(f8c_sb[:], f8c[:])
                    featst = fpool.tile([P, LC, D], F8)
                    nc.sync.dma_start(
                        featst[:],
                        feats[b, h * HALF:(h + 1) * HALF, :]
                        .rearrange("(c p) d -> p c d", p=P))
                    featsts.append(featst)
                    if u == 0:
                        nc.sync.dma_start(f32c_sb[:], f32c[:])
                        nc.sync.dma_start(smallc_sb[:], smallc[:])
                        nc.sync.dma_start(bfc_sb[:], bfc[:])
                for k0 in range(0, N_BF, 2):
                    nc.sync.dma_start(
                        WT_sb[:, k0:k0 + 2, :],
                        WT[k0 * P:(k0 + 2) * P, :]
                        .rearrange("(j p) n -> p j n", p=P))
                for k0 in range(0, N_E3M4, 4):
                    nc.sync.dma_start(
                        WT83_sb[:, k0:k0 + 4, :],
                        WT83[k0 * P:(k0 + 4) * P, :]
                        .rearrange("(j p) n -> p j n", p=P))
                # fc W in 3 gc-block pieces: [i,f], [g], then [o] dead
                # last -- the final-byte -> output chain is just sig_o * t2
                for j0, j1, g0, g1 in ((0, 4, 0, 8), (0, 4, 12, 16),
                                       (0, 2, 8, 12), (2, 4, 8, 12)):
                    nc.sync.dma_start(
                        WT8_sb[:, j0:j1, g0 * P:g1 * P],
                        WT8[j0 * P:j1 * P, g0 * P:g1 * P]
                        .rearrange("(j p) n -> p j n", p=P))

                # Pool-built constants emitted after the DMA stream so they
                # run behind the init barrier instead of gating it
                masks.make_identity(nc, ident[:])
                nc.gpsimd.memset(ones_f8[:], 1.0)

                # ============ phase A: q and beta matvecs (fp8, x1024) ======
                with tc.tile_pool(name="psA", bufs=2, space="PSUM") as psA:
                    for dc in range(DC):
                        qt = psA.tile([P, B_LOC], F32, tag="q")
                        for hc in range(HC):
                            nc.tensor.matmul(
                                qt[:],
                                f8c_sb[:, hc * D + dc * P:
                                       hc * D + (dc + 1) * P],
                                f8c_sb[:, F8C_HT + hc * B_LOC:
                                       F8C_HT + (hc + 1) * B_LOC],
                                start=(hc == 0), stop=(hc == HC - 1))
                        nc.scalar.activation(qb[:, dc, :], qt[:], AF.Identity,
                                             bias=f32c_sb[:, 16 + dc:17 + dc],
                                             scale=1.0 / S_W)
                    bps = psA.tile([1, B_LOC], F32, tag="beta", bufs=1)
                    for hc in range(HC):
                        nc.tensor.matmul(
                            bps[:],
                            f8c_sb[:, F8C_WSEL + hc:F8C_WSEL + hc + 1],
                            f8c_sb[:, F8C_HT + hc * B_LOC:
                                   F8C_HT + (hc + 1) * B_LOC],
                            start=(hc == 0), stop=(hc == HC - 1))
                    # beta = sigmoid(z) = 0.5*tanh(z/2) + 0.5 (one ACT table)
                    nc.scalar.activation(tb[:], bps[:], AF.Tanh,
                                         bias=f32c_sb[0:1, 20:21],
                                         scale=0.5 / S_W)
                    # beta' = 64*sigmoid(z): the 64 is the fp8 fc x-scale
                    nc.vector.tensor_scalar(beta_sb[:], tb[:], 32.0, 32.0,
                                            op0=ALU.mult, op1=ALU.add)

                # ================= phase B: attention + gates + LSTM ========
                with tc.tile_pool(name="pslog", bufs=2, space="PSUM") as pslog, \
                     tc.tile_pool(name="pssum", bufs=1, space="PSUM") as pssum, \
                     tc.tile_pool(name="psctx", bufs=1, space="PSUM") as psctx, \
                     tc.tile_pool(name="psg", bufs=1, space="PSUM") as psg:

                    # gate psums split per block (i|f, o, g) so the LSTM
                    # tail can read each as soon as its own writers finish;
                    # one start=True bias-broadcast preload per tile
                    g_if = psg.tile([P, 32], F32, tag="gif")
                    g_o = psg.tile([P, 16], F32, tag="go")
                    g_g = psg.tile([P, 16], F32, tag="gg")
                    for tile_, c0, c1 in ((g_if, 0, 32), (g_o, 32, 48),
                                          (g_g, 48, 64)):
                        nc.tensor.matmul(
                            tile_[:], smallc_sb[:, 0:P],
                            smallc_sb[:, SC_BRHS + c0:SC_BRHS + c1],
                            start=True, stop=False)

                    def g_dst(gc):
                        if gc < 8:
                            return g_if[:, gc * B_LOC:(gc + 1) * B_LOC]
                        if gc < 12:
                            return g_o[:, (gc - 8) * B_LOC:(gc - 7) * B_LOC]
                        return g_g[:, (gc - 12) * B_LOC:(gc - 11) * B_LOC]

                    fcA = {}
                    for b in range(B_LOC):
                        for h in range(2):
                            u = b * 2 + h
                            r = u
                            projt, featst = projts[u], featsts[u]

                            # relu(proj + q): dc0 on Pool, dc1 on DVE,
                            # dc2/dc3 on ACT (three engines in parallel)
                            hatt = hattp.tile([P, DC, HALF], BF16)
                            nc.gpsimd.tensor_scalar(
                                hatt[:, 0, :], projt[:, 0, :],
                                qb[:, 0, b:b + 1], 0.0,
                                op0=ALU.add, op1=ALU.max)
                            nc.vector.tensor_scalar(
                                hatt[:, 1, :], projt[:, 1, :],
                                qb[:, 1, b:b + 1], 0.0,
                                op0=ALU.add, op1=ALU.max)
                            for dc in (2, 3):
                                nc.scalar.activation(
                                    hatt[:, dc, :], projt[:, dc, :], AF.Relu,
                                    bias=qb[:, dc, b:b + 1])

                            # mask preload -> logitsT psum [128, 8], one start
                            lg_ps = pslog.tile([P, LC], F32)
                            nc.tensor.matmul(
                                lg_ps[:],
                                smallc_sb[0:LC, SC_MADD + r * P:
                                          SC_MADD + (r + 1) * P],
                                smallc_sb[0:LC, SC_ID8:SC_ID8 + LC],
                                start=True, stop=False)
                            # logitsT: stationary hatt chunks, moving w column
                            for dc in range(DC):
                                for lc in range(LC):
                                    nc.tensor.matmul(
                                        lg_ps[:, lc:lc + 1],
                                        hatt[:, dc, lc * P:(lc + 1) * P],
                                        bfc_sb[:, BFC_WPF + dc * 2 + h:
                                               BFC_WPF + dc * 2 + h + 1],
                                        start=False,
                                        stop=(dc == DC - 1 and lc == LC - 1))

                            # softmax: exp on [128, 8]; alpha straight to fp8
                            alpha_t = alphap.tile([P, LC, 1], F8)
                            nc.scalar.activation(alpha_t[:, :, 0], lg_ps[:],
                                                 AF.Exp)
                            sums_ps = pssum.tile([1, LC], F32, tag="sums")
                            nc.tensor.matmul(sums_ps[:], ones_f8[:],
                                             alpha_t[:, :, 0],
                                             start=True, stop=True)
                            nc.vector.tensor_reduce(
                                sums_sb[0:1, r:r + 1], sums_ps[0:1, :],
                                axis=mybir.AxisListType.X, op=ALU.add)
                            nc.vector.reciprocal(recips[0:1, r:r + 1],
                                                 sums_sb[0:1, r:r + 1])
                            nc.vector.tensor_tensor(svals[0:1, r:r + 1],
                                                    recips[0:1, r:r + 1],
                                                    beta_sb[0:1, b:b + 1],
                                                    op=ALU.mult)

                            # ctx[1, 512] += alpha_lc.T @ feats_lc (fp8)
                            ctx_ps = psctx.tile([1, D], F32)
                            for lc in range(LC):
                                nc.tensor.matmul(
                                    ctx_ps[:], alpha_t[:, lc, :],
                                    featst[:, lc, :],
                                    start=(lc == 0), stop=(lc == LC - 1))
                            if h == 0:
                                fcA_b = fcp.tile([1, D], F32, tag="fcA", bufs=2)
                                nc.vector.tensor_scalar_mul(
                                    fcA_b[:], ctx_ps[0:1, :],
                                    svals[0:1, r:r + 1])
                                fcA[b] = fcA_b
                            else:
                                fc_b = fcp.tile([1, D], F32, tag="fcB", bufs=2)
                                nc.vector.scalar_tensor_tensor(
                                    fc_b[:], ctx_ps[0:1, :], svals[0:1, r:r + 1],
                                    fcA[b][:], op0=ALU.mult, op1=ALU.add)
                                for dc in range(DC):
                                    tr_ps = pssum.tile([P, 1], F32, tag="tr",
                                                       bufs=1)
                                    nc.tensor.transpose(
                                        tr_ps[:, 0:1],
                                        fc_b[0:1, dc * P:(dc + 1) * P],
                                        ident[0:1, 0:1])
                                    nc.vector.tensor_copy(
                                        xf8_sb[:, dc * B_LOC + b:
                                               dc * B_LOC + b + 1],
                                        tr_ps[:])

                    # ========== phase C: gates + LSTM tail ==================
                    # gates matmuls in WT-arrival order; fp8 fc chunks (and
                    # the psum stop) land last, as the final W bytes arrive
                    for kc in range(N_BF):
                        for gc in range(GC):
                            nc.tensor.matmul(
                                g_dst(gc),
                                WT_sb[:, kc, gc * P:(gc + 1) * P],
                                bfc_sb[:, kc * B_LOC:(kc + 1) * B_LOC],
                                start=False, stop=False)
                    for j in range(N_E3M4):
                        kc = N_BF + j
                        for gc in range(GC):
                            nc.tensor.matmul(
                                g_dst(gc),
                                WT83_sb[:, j, gc * P:(gc + 1) * P],
                                bfc_sb[:, kc * B_LOC:(kc + 1) * B_LOC],
                                start=False, stop=False)
                    for d0, d1, g0, g1 in ((0, 4, 0, 8), (0, 4, 12, 16),
                                           (0, 2, 8, 12), (2, 4, 8, 12)):
                        for dc in range(d0, d1):
                            for gc in range(g0, g1):
                                nc.tensor.matmul(
                                    g_dst(gc),
                                    WT8_sb[:, dc, gc * P:(gc + 1) * P],
                                    xf8_sb[:, dc * B_LOC:(dc + 1) * B_LOC],
                                    start=False,
                                    stop=(dc == 3 and
                                          gc in (7, 11, 15)))

                    # transposed LSTM tail; gate rows (permuted) = [i,f,o,g]
                    lstm = const
                    t_if = lstm.tile([P, 32], F32)
                    t_o = lstm.tile([P, 16], F32)
                    t_g = lstm.tile([P, 16], F32)
                    t2 = lstm.tile([P, 16], F32)
                    c2 = lstm.tile([P, 16], F32)
                    outt = lstm.tile([P, 32], F32)   # [c_new | h_new]

                    # sigmoid(x) = 0.5*tanh(x/2)+0.5 for i, f, o; tanh
                    # for g; psum carries S_G=256-scaled gates
                    nc.scalar.activation(t_if[:], g_if[:], AF.Tanh,
                                         scale=0.5 / S_G)
                    nc.scalar.activation(t_g[:], g_g[:], AF.Tanh,
                                         scale=1.0 / S_G)
                    nc.vector.tensor_scalar(t_if[:], t_if[:], 0.5, 0.5,
                                            op0=ALU.mult, op1=ALU.add)
                    # c_new = sig_f * c_last + sig_i * tanh_g
                    nc.vector.tensor_tensor(outt[:, 0:16], t_if[:, 16:32],
                                            f32c_sb[:, 0:16], op=ALU.mult)
                    nc.vector.tensor_tensor(c2[:], t_if[:, 0:16], t_g[:],
                                            op=ALU.mult)
                    nc.vector.tensor_tensor(outt[:, 0:16], outt[:, 0:16],
                                            c2[:], op=ALU.add)
                    # o gates land last; t_o goes first on ACT so it is
                    # not queued behind tanh(c_new)
                    nc.scalar.activation(t_o[:], g_o[:], AF.Tanh,
                                         scale=0.5 / S_G)
                    nc.scalar.activation(t2[:], outt[:, 0:16], AF.Tanh)
                    nc.vector.tensor_scalar(t_o[:], t_o[:], 0.5, 0.5,
                                            op0=ALU.mult, op1=ALU.add)
                    nc.vector.tensor_tensor(outt[:, 16:32], t_o[:], t2[:],
                                            op=ALU.mult)
                    nc.sync.dma_start(out_d[:], outt[:])

    nc.compile()
    return nc


_NC_CACHE = None


def _get_nc():
    global _NC_CACHE
    if _NC_CACHE is None:
        _NC_CACHE = build_nc()
    return _NC_CACHE


def split_out(arr):
    """[128, 32] device layout -> (h_new [B_LOC, H], c_new [B_LOC, H])."""
    a = np.asarray(arr, np.float32).reshape(P, 2, HC, B_LOC)
    c = np.ascontiguousarray(a[:, 0].transpose(2, 1, 0).reshape(B_LOC, H))
    h = np.ascontiguousarray(a[:, 1].transpose(2, 1, 0).reshape(B_LOC, H))
    return h, c


def make_in_maps(features, features_proj, hidden_states, cell_states,
                 caption_hidden_states, w_h2a, b_h2a, w_patt, b_patt,
                 w_fatt, b_fatt, w_sel, b_sel, w_ih, w_hh, b_ih, b_hh,
                 mask, feature_idx):
    assert int(feature_idx) == FIDX
    import ml_dtypes
    f32 = np.float32
    bf16 = ml_dtypes.bfloat16
    f8 = ml_dtypes.float8_e4m3
    features = np.asarray(features, f32)
    features_proj = np.asarray(features_proj, f32)
    h_last = np.asarray(hidden_states, f32)[-1]          # [B, H]
    c_last = np.asarray(cell_states, f32)[-1]            # [B, H]
    cap = np.asarray(caption_hidden_states, f32)         # [B, H]
    mask = np.asarray(mask)

    # shared (replicated) tensors — layout-only host prep + dtype casts
    Wfull = np.concatenate([np.asarray(w_ih, f32), np.asarray(w_hh, f32)], axis=1)
    gate_perm = np.r_[0:512, 512:1024, 1536:2048, 1024:1536]   # [i, f, o, g]
    k_perm = np.r_[0:512, 1024:1536, 1536:2048, 512:1024]      # [cap,feat,h,fc]
    f83 = ml_dtypes.float8_e3m4
    b_ihh = (np.asarray(b_ih, f32) + np.asarray(b_hh, f32))[gate_perm] * S_G
    WTf = Wfull[gate_perm][:, k_perm].T                       # [k, g]
    WTh = np.ascontiguousarray(WTf[0:N_BF * P] * S_G).astype(bf16)
    WT83h = np.ascontiguousarray(
        WTf[N_BF * P:12 * P] * S_G).astype(f83)
    WT8h = np.ascontiguousarray(WTf[12 * P:] * (S_G / 64.0)).astype(f8)

    # small bf16 pack rows=16: biasT2 | brhs | ident8 | madd2 (per core)
    sc_base = np.zeros((16, 1224), f32)
    sc_base[:, 0:P] = b_ihh.reshape(GC, P)
    sc_base[:, SC_BRHS:SC_BRHS + GC * B_LOC] = np.kron(
        np.eye(GC, dtype=f32), np.ones((1, B_LOC), f32))
    sc_base[0:LC, SC_ID8:SC_ID8 + LC] = np.eye(LC, dtype=f32)

    # f8 pack: w_h2a (hc-major) | w_sel | hT8 (hT8 filled per core)
    f8c_w = np.zeros((P, 2068), f32)
    w_h2aT = np.asarray(w_h2a, f32).T * S_W                    # [H, D]
    f8c_w[:, 0:2048] = w_h2aT.reshape(HC, P, D).transpose(1, 0, 2) \
        .reshape(P, 2048)
    f8c_w[:, F8C_WSEL:F8C_WSEL + HC] = (
        np.asarray(w_sel, f32).T * S_W).reshape(HC, P).T

    w_pf = np.stack([np.asarray(w_patt, f32)[0], np.asarray(w_fatt, f32)[0]],
                    axis=1)                                    # [D, 2]
    madd = np.where(mask, f32(0), f32(-1e30))                  # [B, L]

    in_maps = []
    for c in range(N_CORES):
        sl = slice(c * B_LOC, (c + 1) * B_LOC)
        sc = sc_base.copy()
        # madd2[lc, (b*2+h)*128 + p] = madd_loc[b, h*HALF + lc*128 + p]
        m = madd[sl].reshape(B_LOC, 2, LC, P)
        sc[0:LC, SC_MADD:SC_MADD + 2 * B_LOC * P] = (
            m.transpose(2, 0, 1, 3).reshape(LC, 2 * B_LOC * P))

        f8cc = f8c_w.copy()
        f8cc[:, F8C_HT:F8C_HT + 16] = (
            h_last[sl].T.reshape(HC, P, B_LOC).transpose(1, 0, 2)
            .reshape(P, 16))

        bfc = np.zeros((P, 72), f32)
        xh = np.stack([cap[sl], features[sl, FIDX, :], h_last[sl]], axis=0)
        # xh [3, B_LOC, H] -> bfc[p, kc*4+b] for kc blocks cap/feat/h
        bfc[:, 0:48] = xh.reshape(3, B_LOC, HC, P).transpose(3, 0, 2, 1) \
            .reshape(P, 48)
        bfc[:, BFC_WPF:BFC_WPF + 8] = w_pf.reshape(DC, P, 2) \
            .transpose(1, 0, 2).reshape(P, 8)

        f32cc = np.zeros((P, 21), f32)
        f32cc[:, 0:16] = c_last[sl].T.reshape(HC, P, B_LOC) \
            .transpose(1, 0, 2).reshape(P, 16)
        f32cc[:, 16:20] = np.asarray(b_h2a, f32).reshape(DC, P).T
        f32cc[0, 20] = np.asarray(b_sel, f32).reshape(-1)[0] * 0.5

        in_maps.append({
            "projT": np.ascontiguousarray(
                features_proj[sl].transpose(0, 2, 1)).astype(f8),
            "feats": np.ascontiguousarray(features[sl]).astype(f8),
            "WT": WTh,
            "WT83": WT83h,
            "WT8": WT8h,
            "f8c": f8cc.astype(f8),
            "f32c": f32cc,
            "bfc": bfc.astype(bf16),
            "smallc": sc.astype(bf16),
        })
    return in_maps


def run(trace=False, **inputs):
    nc = _get_nc()
    in_maps = make_in_maps(**inputs)
    res = run_bass_kernel_spmd(nc, in_maps, core_ids=list(range(N_CORES)),
                               trace=trace)
    hs, cs = [], []
    for c in range(N_CORES):
        h, cc = split_out(res.results[c]["out"])
        hs.append(h)
        cs.append(cc)
    return (np.concatenate(hs)[None], np.concatenate(cs)[None]), res


def kernel(**inputs):
    out, _ = run(trace=False, **inputs)
    return out
